# revision 1
# baseline (speedup 1.0000x reference)
"""Multi-head attention + residual + layernorm kernel for 8 Trainium2 cores.

Reference computation (B=4, S=2048, D=1024, H=16, dk=64):
    qh,kh,vh = split_heads(x @ W{q,k,v}.T + b)   per batch
    attn     = softmax(qh @ kh^T / 8) @ vh       (mask all-ones)
    out      = LN(concat(attn) @ Wo.T + bo + q)

Sharding: core c -> (batch b = c//2, query rows half = c%2). Each core
computes all 16 heads for its 1024 query rows, using the full 2048 K/V
rows of its batch. No collectives; host concatenates the 8 output shards.

On-device layout strategy (everything PE-friendly, zero device transposes):
  - host uploads q/k/v transposed (qT/kT/vT: [din, s]) + W.T (WT: [din, dout])
  - Q,K projections computed transposed: qhT/khT [dout, s]
  - V projection computed natural: vh [s, dout]
  - scores^T [keys, q] = khT_slice.T @ qhT_slice   (K = dk = 64; head A on
    partitions 0:64, head B on 64:128 -> concurrent PE row groups)
  - exp on ACT with fused 1/8 scale, no max subtraction (scores are O(10);
    fp32 exp cannot overflow, softmax is shift-invariant)
  - PV matmul lhsT = [vh | ones] (head A) / [ones | vh] (head B): one
    accumulating matmul yields attn^T rows AND the softmax denominator
    on the complementary 64 partitions.
  - attn^T [dk, q] slices feed the out-projection directly as stationary
    operands; out [s, dout] lands in natural layout for LN + store.
"""

import numpy as np

import concourse.bass as bass
import concourse.mybir as mybir
import concourse.tile as tile
from concourse import bacc
from concourse.bass_utils import run_bass_kernel_spmd

F32 = mybir.dt.float32
AF = mybir.ActivationFunctionType

B, S, D, H = 4, 2048, 1024, 16
DK = D // H          # 64
NCORES = 8
SQ = S // 2          # query rows per core = 1024
NPAIR = 8            # head pairs; pair p = heads (2p, 2p+1), douts 128p..+128
CH = D // 128        # 8 contraction chunks of 128
LNEPS = 1e-5

# matmul input dtype: float32r streams 1 col/cycle (vs 4 for float32) at N>=256
MM_DT = mybir.dt.float32r




def build_core_program(nc, sq=SQ, skv=S, repeat=1, phases='ABC'):
    """Emit the per-core program. sq/skv parameterized only for mini-tests."""
    kcn = skv // 128      # PV contraction chunks (16)
    n_sq_t = sq // 512    # q 512-tiles for Q projection (2)
    n_skv_t = skv // 512  # s 512-tiles for K projection (4)
    n_vs_t = skv // 128   # s 128-tiles for V projection (16)
    n_st = sq // 128      # out s-tiles (8)
    nqt = sq // 512       # q 512-tiles inside attention (2)

    def din(name, shape, dt=F32):
        return nc.dram_tensor(name, shape, dt, kind="ExternalInput").ap()

    qT = din("qT", [D, sq], MM_DT)     # q rows of this core, transposed
    kT = din("kT", [D, skv], MM_DT)
    vT = din("vT", [D, skv], MM_DT)
    wqT = din("wqT", [D, D], MM_DT)    # Wq.T etc. ([din, dout])
    wkT = din("wkT", [D, D], MM_DT)
    wvT = din("wvT", [D, D], MM_DT)
    woT = din("woT", [D, D], MM_DT)
    bq = din("bq", [D])
    bk = din("bk", [D])
    bv = din("bv", [D])
    resid = din("resid", [sq, D])  # q rows + bo (host precomputed)
    lng = din("lng", [D])
    lnb = din("lnb", [D])
    out = nc.dram_tensor("out", [sq, D], F32, kind="ExternalOutput").ap()

    with tile.TileContext(nc) as tc:
        with (
            tc.tile_pool(name="dram", bufs=1, space="DRAM") as dram,
            tc.tile_pool(name="weights", bufs=1) as weights,
            tc.tile_pool(name="acts", bufs=2) as acts,
            tc.tile_pool(name="consts", bufs=1) as consts,
            tc.tile_pool(name="projout", bufs=2) as projout,
            tc.tile_pool(name="attn_in", bufs=2) as attn_in,
            tc.tile_pool(name="va_pool", bufs=3) as va_pool,
            tc.tile_pool(name="exps", bufs=4) as exps,
            tc.tile_pool(name="attnT", bufs=NPAIR) as attnT_pool,
            tc.tile_pool(name="eptmp", bufs=1) as eptmp,
            tc.tile_pool(name="xtiles", bufs=3) as xtiles,
            tc.tile_pool(name="stats", bufs=4) as stats_pool,
        ):
            # staging in DRAM
            kht_st = dram.tile([NPAIR, 128, skv], MM_DT)
            vh_st = dram.tile([skv, D], MM_DT)             # [s, dout]

            # per-dout bias, striped so dout = pair*128 + p -> [p, pair]
            bq_sb = consts.tile([128, NPAIR], F32)
            nc.scalar.dma_start(bq_sb, bq.rearrange("(pr p) -> p pr", p=128))
            bk_sb = consts.tile([128, NPAIR], F32)
            nc.scalar.dma_start(bk_sb, bk.rearrange("(pr p) -> p pr", p=128))
            # bv broadcast across partitions ([dout] lives on free dim for vh)
            bv_sb = consts.tile([128, D], F32)
            nc.scalar.dma_start(bv_sb, bv[None, :].to_broadcast((128, D)))
            lng_sb = consts.tile([128, D], F32)
            nc.scalar.dma_start(lng_sb, lng[None, :].to_broadcast((128, D)))
            lnb_sb = consts.tile([128, D], F32)
            nc.scalar.dma_start(lnb_sb, lnb[None, :].to_broadcast((128, D)))
            eps_sb = consts.tile([128, 1], F32)
            nc.vector.memset(eps_sb, LNEPS)
            ones_sb = consts.tile([128, DK], F32)
            nc.vector.memset(ones_sb, 1.0)

            for _rep in range(repeat):
             with tc.tile_pool(name=f"psum{_rep}", bufs=2, space="PSUM") \
                     as psum_pool:
              def proj_ps():
                  return psum_pool.tile([128, sq], F32, tag="pv", name="projps")[:, 0:512]

              # ---- Phase A: projections ------------------------------
              if "A" not in phases:
                  continue
              if True:
                  # A1: K projection -> kht_st ([dout, s], accumulate din chunks)
                  wk_sb = weights.tile([128, CH, D], MM_DT, tag="w")
                  nc.sync.dma_start(wk_sb, wkT.rearrange("(c p) m -> p c m", p=128))
                  for st in range(n_skv_t):
                      kt_sb = acts.tile([128, CH, 512], MM_DT, tag="act")
                      nc.scalar.dma_start(
                          kt_sb,
                          kT.rearrange("(c p) s -> p c s", p=128)[
                              :, :, st * 512:(st + 1) * 512],
                      )
                      for pr in range(NPAIR):
                          ps = proj_ps()
                          for c in range(CH):
                              nc.tensor.matmul(
                                  ps,
                                  lhsT=(wk_sb[:, c, pr * 128:(pr + 1) * 128]),
                                  rhs=(kt_sb[:, c, :]),
                                  start=(c == 0),
                                  stop=(c == CH - 1),
                              )
                          o_sb = projout.tile([128, 512], MM_DT, tag="po")
                          nc.vector.tensor_scalar_add(
                              o_sb, ps, scalar1=bk_sb[:, pr:pr + 1])
                          nc.gpsimd.dma_start(
                              kht_st[pr, :, st * 512:(st + 1) * 512], o_sb)

                  # A2: V projection -> vh_st (natural [s, dout])
                  wv_sb = weights.tile([128, CH, D], MM_DT, tag="w")
                  nc.sync.dma_start(wv_sb, wvT.rearrange("(c p) m -> p c m", p=128))
                  for st in range(n_vs_t):
                      vt_sb = acts.tile([128, CH, 128], MM_DT, tag="act")
                      nc.scalar.dma_start(
                          vt_sb,
                          vT.rearrange("(c p) s -> p c s", p=128)[
                              :, :, st * 128:(st + 1) * 128],
                      )
                      for dt in range(2):
                          ps = proj_ps()
                          for c in range(CH):
                              nc.tensor.matmul(
                                  ps,
                                  lhsT=(vt_sb[:, c, :]),
                                  rhs=(wv_sb[:, c, dt * 512:(dt + 1) * 512]),
                                  start=(c == 0),
                                  stop=(c == CH - 1),
                              )
                          o_sb = projout.tile([128, 512], MM_DT, tag="po")
                          nc.vector.tensor_add(
                              o_sb, ps, bv_sb[:, dt * 512:(dt + 1) * 512])
                          nc.gpsimd.dma_start(
                              vh_st[st * 128:(st + 1) * 128,
                                    dt * 512:(dt + 1) * 512], o_sb)

                  # A3: Q projection -> qht tiles in the attnT pool
                  # ([dout, s] layout; the pool's 8 slots hold qhT until each
                  # pair's scores consume it, then rotate into attnT storage)
                  qht_tiles = []
                  wq_sb = weights.tile([128, CH, D], MM_DT, tag="w")
                  nc.sync.dma_start(wq_sb, wqT.rearrange("(c p) m -> p c m", p=128))
                  qt_sbs = []
                  for st in range(n_sq_t):
                      qt_sb = acts.tile([128, CH, 512], MM_DT, tag="act")
                      nc.scalar.dma_start(
                          qt_sb,
                          qT.rearrange("(c p) s -> p c s", p=128)[
                              :, :, st * 512:(st + 1) * 512],
                      )
                      qt_sbs.append(qt_sb)
                  for pr in range(NPAIR):
                      qh_t = attnT_pool.tile([128, sq], MM_DT, tag="attnT",
                                             name="qht")
                      qht_tiles.append(qh_t)
                      for st in range(n_sq_t):
                          ps = proj_ps()
                          for c in range(CH):
                              nc.tensor.matmul(
                                  ps,
                                  lhsT=(wq_sb[:, c, pr * 128:(pr + 1) * 128]),
                                  rhs=(qt_sbs[st][:, c, :]),
                                  start=(c == 0),
                                  stop=(c == CH - 1),
                              )
                          nc.vector.tensor_scalar_add(
                              qh_t[:, st * 512:(st + 1) * 512], ps,
                              scalar1=bq_sb[:, pr:pr + 1])

              # out-projection weights (phase C; prefetch after wq frees)
              wo_sb = weights.tile([128, CH, D], MM_DT, tag="w")
              nc.sync.dma_start(wo_sb, woT.rearrange("(c p) m -> p c m", p=128))

              # ---- Phase B: attention per head-pair --------------------
              if "B" not in phases:
                  continue
              attnT = []
              if True:
                  spsum = pvpsum = psum_pool
                  for pr in range(NPAIR):
                      kh_sb = attn_in.tile([128, skv], MM_DT, tag="kh")
                      nc.gpsimd.dma_start(kh_sb, kht_st[pr])
                      qh_sb = qht_tiles[pr]
                      # augmented PV stationary tiles:
                      # head A (even): [vh | ones] -> rows 0:64 attnT, 64:128 sum
                      # head B (odd):  [ones | vh] -> rows 0:64 sum, 64:128 attnT
                      vaA = va_pool.tile([128, kcn, 128], MM_DT, tag="va")
                      nc.gpsimd.dma_start(
                          vaA[:, :, 0:DK],
                          vh_st[:, 128 * pr:128 * pr + DK].rearrange(
                              "(kc p) d -> p kc d", p=128),
                      )
                      nc.vector.tensor_copy(
                          out=vaA[:, :, DK:128],
                          in_=ones_sb[:, None, :].to_broadcast((128, kcn, DK)))
                      vaB = va_pool.tile([128, kcn, 128], MM_DT, tag="va")
                      nc.gpsimd.dma_start(
                          vaB[:, :, DK:128],
                          vh_st[:, 128 * pr + DK:128 * pr + 128].rearrange(
                              "(kc p) d -> p kc d", p=128),
                      )
                      nc.vector.tensor_copy(
                          out=vaB[:, :, 0:DK],
                          in_=ones_sb[:, None, :].to_broadcast((128, kcn, DK)))

                      pvA = pvpsum.tile([128, sq], F32, tag="pv")
                      pvB = pvpsum.tile([128, sq], F32, tag="pv")

                      # software-pipelined: scores(kc+1) is emitted BEFORE
                      # PV(kc) so the in-order PE queue never stalls behind a
                      # PV matmul that waits on exp(kc) (ACT); steady state
                      # runs PE [scores(kc+1), PV(kc)] || ACT [exp(kc)].
                      def emit_scores(kc):
                          ksl = slice(kc * 128, (kc + 1) * 128)
                          sc = spsum.tile([128, sq], F32, tag="sc", name="sc")
                          scB = spsum.tile([128, sq], F32, tag="sc", name="scB")
                          for qt in range(nqt):
                              qs = slice(qt * 512, (qt + 1) * 512)
                              # head A (rows 0:64) and head B (rows 64:128)
                              # land on different PE row groups -> concurrent
                              nc.tensor.matmul(
                                  sc[:, qs],
                                  lhsT=(kh_sb[0:DK, ksl]),
                                  rhs=(qh_sb[0:DK, qs]),
                                  start=True, stop=True,
                              )
                              nc.tensor.matmul(
                                  scB[:, qs],
                                  lhsT=(kh_sb[DK:128, ksl]),
                                  rhs=(qh_sb[DK:128, qs]),
                                  start=True, stop=True,
                              )
                          return sc, scB

                      sc_next = emit_scores(0)
                      for kc in range(kcn):
                          sc, scB = sc_next
                          sc_next = emit_scores(kc + 1) if kc + 1 < kcn else None
                          ex = exps.tile([128, sq], MM_DT, tag="ex", name="ex")
                          exB = exps.tile([128, sq], MM_DT, tag="ex", name="exB")
                          nc.scalar.activation(ex, sc, AF.Exp, scale=1.0 / np.sqrt(DK))
                          nc.scalar.activation(exB, scB, AF.Exp, scale=1.0 / np.sqrt(DK))
                          for qt in range(nqt):
                              qs = slice(qt * 512, (qt + 1) * 512)
                              nc.tensor.matmul(
                                  pvA[:, qs], lhsT=(vaA[:, kc, :]),
                                  rhs=(ex[:, qs]),
                                  start=(kc == 0), stop=(kc == kcn - 1),
                              )
                              nc.tensor.matmul(
                                  pvB[:, qs], lhsT=(vaB[:, kc, :]),
                                  rhs=(exB[:, qs]),
                                  start=(kc == 0), stop=(kc == kcn - 1),
                              )

                      # epilogue: attnT[0:64] = pvA[0:64] * 1/sumA (sumA on
                      # pvA[64:128]); attnT[64:128] = pvB[64:128] * 1/sumB
                      at = attnT_pool.tile([128, sq], MM_DT, tag="attnT",
                                           name="attnT")
                      attnT.append(at)
                      rt = eptmp.tile([128, sq], F32, tag="rt", name="rt")
                      nc.vector.reciprocal(rt[64:128, :], pvA[64:128, :])
                      nc.vector.reciprocal(rt[0:64, :], pvB[0:64, :])
                      rs = eptmp.tile([128, sq], F32, tag="rs", name="rs")
                      nc.gpsimd.dma_start(rs[0:64, :], rt[64:128, :])   # shift
                      nc.gpsimd.dma_start(rs[64:128, :], rt[0:64, :])   # shift
                      nc.vector.tensor_mul(at[0:64, :], pvA[0:64, :], rs[0:64, :])
                      nc.vector.tensor_mul(
                          at[64:128, :], pvB[64:128, :], rs[64:128, :])

              # ---- Phase C: out projection + residual + layernorm ------
              if "C" not in phases:
                  continue
              if True:
                  for st in range(n_st):
                      ss = slice(st * 128, (st + 1) * 128)
                      x_sb = xtiles.tile([128, D], F32, tag="x")
                      nc.gpsimd.dma_start(x_sb, resid[ss, :])
                      for dt in range(2):
                          ps = proj_ps()
                          for pr in range(NPAIR):
                              nc.tensor.matmul(
                                  ps,
                                  lhsT=(attnT[pr][:, ss]),
                                  rhs=(wo_sb[:, pr, dt * 512:(dt + 1) * 512]),
                                  start=(pr == 0),
                                  stop=(pr == NPAIR - 1),
                              )
                          dsl = slice(dt * 512, (dt + 1) * 512)
                          nc.vector.tensor_add(x_sb[:, dsl], ps, x_sb[:, dsl])
                      # layernorm over D (free dim)
                      stt = stats_pool.tile([128, 2, 6], F32, tag="bst")
                      nc.vector.bn_stats(stt[:, 0, :], x_sb[:, 0:512])
                      nc.vector.bn_stats(stt[:, 1, :], x_sb[:, 512:1024])
                      mv = stats_pool.tile([128, 2], F32, tag="mv")
                      nc.vector.bn_aggr(mv, stt)
                      std = stats_pool.tile([128, 1], F32, tag="std")
                      nc.scalar.activation(
                          std, mv[:, 1:2], AF.Sqrt, bias=eps_sb[:, 0:1])
                      rstd = stats_pool.tile([128, 1], F32, tag="rstd")
                      nc.vector.reciprocal(rstd, std)
                      nc.vector.tensor_scalar(
                          x_sb, x_sb,
                          scalar1=mv[:, 0:1], scalar2=rstd,
                          op0=mybir.AluOpType.subtract, op1=mybir.AluOpType.mult,
                      )
                      nc.vector.tensor_mul(x_sb, x_sb, lng_sb)
                      nc.vector.tensor_add(x_sb, x_sb, lnb_sb)
                      nc.gpsimd.dma_start(out[ss, :], x_sb)

    return nc


_CACHED = {}


def _get_program(sq=SQ, skv=S, repeat=1, phases="ABC"):
    key = (sq, skv, repeat, phases)
    if key not in _CACHED:
        nc = bacc.Bacc("TRN2", target_bir_lowering=False, debug=False)
        build_core_program(nc, sq, skv, repeat, phases)
        nc.finalize()
        _CACHED[key] = nc
    return _CACHED[key]


def make_in_maps(q, k, v, Wq, bq, Wk, bk, Wv, bv, Wo, bo, ln_g, ln_b):
    f = np.float32
    shared = {
        "wqT": np.ascontiguousarray(Wq.T, f),
        "wkT": np.ascontiguousarray(Wk.T, f),
        "wvT": np.ascontiguousarray(Wv.T, f),
        "woT": np.ascontiguousarray(Wo.T, f),
        "bq": np.ascontiguousarray(bq, f),
        "bk": np.ascontiguousarray(bk, f),
        "bv": np.ascontiguousarray(bv, f),
        "lng": np.ascontiguousarray(ln_g, f),
        "lnb": np.ascontiguousarray(ln_b, f),
    }
    in_maps = []
    for c in range(NCORES):
        b, half = c // 2, c % 2
        rows = slice(half * SQ, (half + 1) * SQ)
        in_maps.append({
            **shared,
            "qT": np.ascontiguousarray(q[b, rows, :].T, f),
            "kT": np.ascontiguousarray(k[b].T, f),
            "vT": np.ascontiguousarray(v[b].T, f),
            "resid": np.ascontiguousarray(q[b, rows, :] + bo[None, :], f),
        })
    return in_maps


def kernel(q, k, v, mask, Wq, bq, Wk, bk, Wv, bv, Wo, bo, ln_g, ln_b):
    nc = _get_program()
    in_maps = make_in_maps(q, k, v, Wq, bq, Wk, bk, Wv, bv, Wo, bo, ln_g, ln_b)
    res = run_bass_kernel_spmd(nc, in_maps, core_ids=list(range(NCORES)))
    out = np.empty((B, S, D), np.float32)
    for c in range(NCORES):
        b, half = c // 2, c % 2
        out[b, half * SQ:(half + 1) * SQ, :] = res.results[c]["out"]
    return out



# revision 9
# speedup vs baseline: 1.5396x; 1.5396x over previous
"""Multi-head attention + residual + layernorm kernel for 8 Trainium2 cores.

Reference computation (B=4, S=2048, D=1024, H=16, dk=64):
    qh,kh,vh = split_heads(x @ W{q,k,v}.T + b)   per batch
    attn     = softmax(qh @ kh^T / 8) @ vh       (mask all-ones)
    out      = LN(concat(attn) @ Wo.T + bo + q)

Sharding: core c -> (batch b = c//2, query rows half = c%2). Each core
computes all 16 heads for its 1024 query rows, using the full 2048 K/V
rows of its batch. No collectives; host concatenates the 8 output shards.

v2 design (vs the phase-serial v1):
  - all matmul operands in bf16 (same PE rate as fp32r, half the DMA/SBUF)
  - khT / vh / va live entirely in SBUF: no DRAM staging round-trip
  - K/Q projections are emitted as "proj blocks" interleaved into the
    attention kc-loop of the PREVIOUS pair, filling the PE gaps that the
    ACT-paced softmax leaves; V projection runs in two dout-half passes
    feeding pairs 0-3 / 4-7.
  - PSUM: scores + proj share one 2-slot tag (4 banks), PV holds the
    other 4 banks -> exactly 8.
  - attention per pair: scores^T via kh/qh partition-split (heads A/B on
    PE row groups 0:64 / 64:128), exp on ACT with fused 1/8 scale, PV
    with [vh | ones] augmented stationaries giving attn^T and the
    softmax denominator in one accumulation.
"""

from collections import deque

import numpy as np

import concourse.bass as bass
import concourse.mybir as mybir
import concourse.tile as tile
from concourse import bacc
from concourse.bass_utils import run_bass_kernel_spmd

F32 = mybir.dt.float32
BF16 = mybir.dt.bfloat16
AF = mybir.ActivationFunctionType

B, S, D, H = 4, 2048, 1024, 16
DK = D // H          # 64
NCORES = 8
SQ = S // 2          # query rows per core = 1024
NPAIR = 8            # head pairs; pair p = heads (2p, 2p+1), douts 128p..+128
CH = D // 128        # 8 contraction chunks of 128
LNEPS = 1e-5


def build_core_program(nc, sq=SQ, skv=S, repeat=1, phases='ABC'):
    """Emit the per-core program. sq/skv parameterized only for mini-tests."""
    kcn = skv // 128      # attention key chunks (16)
    n_sq_t = sq // 512    # q 512-tiles (2)
    n_skv_t = skv // 512  # kv 512-tiles (4)
    n_vs_t = skv // 128   # v s-chunks of 128 (16)
    n_st = sq // 128      # out s-tiles (8)
    nqt = sq // 512       # q 512-tiles inside attention (2)

    def din(name, shape, dt=F32):
        return nc.dram_tensor(name, shape, dt, kind="ExternalInput").ap()

    qT = din("qT", [D, sq], BF16)      # this core's q rows, transposed
    kT = din("kT", [D, skv], BF16)
    vT = din("vT", [D, skv], BF16)
    wqT = din("wqT", [D, D], BF16)     # Wq.T etc. ([din, dout])
    wkT = din("wkT", [D, D], BF16)
    wvT = din("wvT", [D, D], BF16)
    woT = din("woT", [D, D], BF16)
    bq = din("bq", [D])
    bk = din("bk", [D])
    bv = din("bv", [D])
    resid = din("resid", [sq, D])  # q rows + bo (host precomputed)
    lng = din("lng", [D])
    lnb = din("lnb", [D])
    out = nc.dram_tensor("out", [sq, D], F32, kind="ExternalOutput").ap()

    with tile.TileContext(nc) as tc:
        with (
            tc.tile_pool(name="consts", bufs=1) as consts,
            tc.tile_pool(name="weights", bufs=1) as weights,
            tc.tile_pool(name="acts", bufs=1) as acts,
            tc.tile_pool(name="vtp", bufs=2) as vtp,
            tc.tile_pool(name="vh", bufs=2) as vh_pool,
            tc.tile_pool(name="khT", bufs=2) as khT_pool,
            tc.tile_pool(name="qht", bufs=2) as qht_pool,
            tc.tile_pool(name="va_pool", bufs=3) as va_pool,
            tc.tile_pool(name="exps", bufs=3) as exps,
            tc.tile_pool(name="attnT", bufs=NPAIR) as attnT_pool,
            tc.tile_pool(name="eptmp", bufs=1) as eptmp,
            tc.tile_pool(name="xtiles", bufs=2) as xtiles,
            tc.tile_pool(name="stats", bufs=4) as stats_pool,
        ):
            # ---- constants -----------------------------------------
            # per-dout bias, striped so dout = pair*128 + p -> [p, pair]
            bq_sb = consts.tile([128, NPAIR], F32)
            nc.scalar.dma_start(bq_sb, bq.rearrange("(pr p) -> p pr", p=128))
            bk_sb = consts.tile([128, NPAIR], F32)
            nc.scalar.dma_start(bk_sb, bk.rearrange("(pr p) -> p pr", p=128))
            bv_sb = consts.tile([128, D], F32)
            nc.scalar.dma_start(bv_sb, bv[None, :].to_broadcast((128, D)))
            lng_sb = consts.tile([128, D], F32)
            nc.scalar.dma_start(lng_sb, lng[None, :].to_broadcast((128, D)))
            lnb_sb = consts.tile([128, D], F32)
            nc.scalar.dma_start(lnb_sb, lnb[None, :].to_broadcast((128, D)))
            eps_sb = consts.tile([128, 1], F32)
            nc.vector.memset(eps_sb, LNEPS)
            ones_sb = consts.tile([128, DK], BF16)
            nc.vector.memset(ones_sb, 1.0)

            for _rep in range(repeat):
             with tc.tile_pool(name=f"psum{_rep}", bufs=2, space="PSUM") \
                     as psum_pool:
              def proj_ps():
                  return psum_pool.tile([128, sq], F32, tag="sc",
                                        name="projps")[:, 0:512]

              # ---- bulk input loads ---------------------------------
              wk_sb = weights.tile([128, CH, D], BF16, tag="wk")
              nc.sync.dma_start(wk_sb, wkT.rearrange("(c p) m -> p c m", p=128))
              kt_sb = acts.tile([128, CH, skv], BF16, tag="kt")
              nc.scalar.dma_start(kt_sb, kT.rearrange("(c p) s -> p c s", p=128))
              wq_sb = weights.tile([128, CH, D], BF16, tag="wq")
              nc.sync.dma_start(wq_sb, wqT.rearrange("(c p) m -> p c m", p=128))
              qt_sb = acts.tile([128, CH, sq], BF16, tag="qt")
              nc.scalar.dma_start(qt_sb, qT.rearrange("(c p) s -> p c s", p=128))
              wv_sb = weights.tile([128, CH, D], BF16, tag="wvo")
              nc.sync.dma_start(wv_sb, wvT.rearrange("(c p) m -> p c m", p=128))

              # ---- proj building blocks -----------------------------
              khT_tiles = [None] * NPAIR
              qht_tiles = [None] * NPAIR
              vh_half = [None, None]  # [128 key-part, kcn, 512 douts] bf16

              def kproj_block(pr, st):
                  def emit():
                      ps = proj_ps()
                      for c in range(CH):
                          nc.tensor.matmul(
                              ps,
                              lhsT=(wk_sb[:, c, pr * 128:(pr + 1) * 128]),
                              rhs=(kt_sb[:, c, st * 512:(st + 1) * 512]),
                              start=(c == 0), stop=(c == CH - 1),
                          )
                      nc.vector.tensor_scalar_add(
                          khT_tiles[pr][:, st * 512:(st + 1) * 512], ps,
                          scalar1=bk_sb[:, pr:pr + 1])
                  return emit

              def qproj_block(pr, st):
                  def emit():
                      ps = proj_ps()
                      for c in range(CH):
                          nc.tensor.matmul(
                              ps,
                              lhsT=(wq_sb[:, c, pr * 128:(pr + 1) * 128]),
                              rhs=(qt_sb[:, c, st * 512:(st + 1) * 512]),
                              start=(c == 0), stop=(c == CH - 1),
                          )
                      nc.vector.tensor_scalar_add(
                          qht_tiles[pr][:, st * 512:(st + 1) * 512], ps,
                          scalar1=bq_sb[:, pr:pr + 1])
                  return emit

              def vproj_block(st, dt):
                  def emit():
                      vt_sb = vtp.tile([128, CH, 128], BF16, tag="vt")
                      nc.sync.dma_start(
                          vt_sb,
                          vT.rearrange("(c p) s -> p c s", p=128)[
                              :, :, st * 128:(st + 1) * 128],
                      )
                      ps = proj_ps()
                      for c in range(CH):
                          nc.tensor.matmul(
                              ps,
                              lhsT=(vt_sb[:, c, :]),
                              rhs=(wv_sb[:, c, dt * 512:(dt + 1) * 512]),
                              start=(c == 0), stop=(c == CH - 1),
                          )
                      nc.vector.tensor_add(
                          vh_half[dt][:, st, :], ps,
                          bv_sb[:, dt * 512:(dt + 1) * 512])
                  return emit

              def emit_kq(pr):
                  khT_tiles[pr] = khT_pool.tile([128, skv], BF16, tag="khT",
                                                name="khT")
                  qht_tiles[pr] = qht_pool.tile([128, sq], BF16, tag="qht",
                                                name="qht")
                  blocks = [kproj_block(pr, st) for st in range(n_skv_t)]
                  blocks += [qproj_block(pr, st) for st in range(n_sq_t)]
                  return blocks

              def emit_vhalf(dt):
                  vh_half[dt] = vh_pool.tile([128, kcn, 512], BF16,
                                             tag="vh", name="vh")
                  return [vproj_block(st, dt) for st in range(n_vs_t)]

              # work queue of pending proj blocks, drained inside the
              # attention kc loops to fill PE gaps
              pending = deque()

              def pop_blocks(n):
                  for _ in range(n):
                      if pending:
                          pending.popleft()()

              # prefix: pair 0's K/Q proj + V douts 0:512 run up front
              if "A" in phases:
                  for b_ in emit_kq(0):
                      b_()
                  for b_ in emit_vhalf(0):
                      b_()

              # ---- attention per head-pair --------------------------
              attnT = []
              wo_sb = None
              if "B" in phases:
                  for pr in range(NPAIR):
                      if pr + 1 < NPAIR and "A" in phases:
                          pending.extend(emit_kq(pr + 1))
                      if pr == 1 and "A" in phases:
                          pending.extend(emit_vhalf(1))
                      if pr == 5:
                          # out-projection weights (reuses wv's slot, which
                          # frees once the last vproj block has run)
                          wo_sb = weights.tile([128, CH, D], BF16, tag="wvo")
                          nc.sync.dma_start(
                              wo_sb,
                              woT.rearrange("(c p) m -> p c m", p=128))

                      kh_sb = khT_tiles[pr]
                      qh_sb = qht_tiles[pr]
                      vh = vh_half[pr // 4]
                      off = (pr % 4) * 128
                      # augmented PV stationary tiles:
                      # head A (even): [vh | ones] -> rows 0:64 attnT, 64:128 sum
                      # head B (odd):  [ones | vh] -> rows 0:64 sum, 64:128 attnT
                      vaA = va_pool.tile([128, kcn, 128], BF16, tag="va")
                      nc.vector.tensor_copy(
                          out=vaA[:, :, 0:DK], in_=vh[:, :, off:off + DK])
                      nc.vector.tensor_copy(
                          out=vaA[:, :, DK:128],
                          in_=ones_sb[:, None, :].to_broadcast((128, kcn, DK)))
                      vaB = va_pool.tile([128, kcn, 128], BF16, tag="va")
                      nc.vector.tensor_copy(
                          out=vaB[:, :, DK:128],
                          in_=vh[:, :, off + DK:off + 128])
                      nc.vector.tensor_copy(
                          out=vaB[:, :, 0:DK],
                          in_=ones_sb[:, None, :].to_broadcast((128, kcn, DK)))

                      pvA = psum_pool.tile([128, sq], F32, tag="pv")
                      pvB = psum_pool.tile([128, sq], F32, tag="pv")

                      # software-pipelined: scores(kc+1) is emitted BEFORE
                      # PV(kc) so the in-order PE queue never stalls behind a
                      # PV matmul that waits on exp(kc) (ACT); steady state
                      # runs PE [scores(kc+1), PV(kc)] || ACT [exp(kc)].
                      def emit_scores(kc):
                          ksl = slice(kc * 128, (kc + 1) * 128)
                          sc = psum_pool.tile([128, sq], F32, tag="sc",
                                              name="sc")
                          scB = psum_pool.tile([128, sq], F32, tag="sc",
                                               name="scB")
                          for qt in range(nqt):
                              qs = slice(qt * 512, (qt + 1) * 512)
                              # head A (rows 0:64) and head B (rows 64:128)
                              # land on different PE row groups -> concurrent
                              nc.tensor.matmul(
                                  sc[:, qs],
                                  lhsT=(kh_sb[0:DK, ksl]),
                                  rhs=(qh_sb[0:DK, qs]),
                                  start=True, stop=True,
                              )
                              nc.tensor.matmul(
                                  scB[:, qs],
                                  lhsT=(kh_sb[DK:128, ksl]),
                                  rhs=(qh_sb[DK:128, qs]),
                                  start=True, stop=True,
                              )
                          return sc, scB

                      sc_next = emit_scores(0)
                      for kc in range(kcn):
                          sc, scB = sc_next
                          sc_next = emit_scores(kc + 1) if kc + 1 < kcn else None
                          ex = exps.tile([128, sq], BF16, tag="ex", name="ex")
                          exB = exps.tile([128, sq], BF16, tag="ex", name="exB")
                          nc.scalar.activation(ex, sc, AF.Exp,
                                               scale=1.0 / np.sqrt(DK))
                          nc.scalar.activation(exB, scB, AF.Exp,
                                               scale=1.0 / np.sqrt(DK))
                          for qt in range(nqt):
                              qs = slice(qt * 512, (qt + 1) * 512)
                              nc.tensor.matmul(
                                  pvA[:, qs], lhsT=(vaA[:, kc, :]),
                                  rhs=(ex[:, qs]),
                                  start=(kc == 0), stop=(kc == kcn - 1),
                              )
                              nc.tensor.matmul(
                                  pvB[:, qs], lhsT=(vaB[:, kc, :]),
                                  rhs=(exB[:, qs]),
                                  start=(kc == 0), stop=(kc == kcn - 1),
                              )
                          pop_blocks(1 + (len(pending) > 6))

                      # epilogue: attnT[0:64] = pvA[0:64] * 1/sumA (sumA on
                      # pvA[64:128]); attnT[64:128] = pvB[64:128] * 1/sumB
                      at = attnT_pool.tile([128, sq], BF16, tag="attnT",
                                           name="attnT")
                      attnT.append(at)
                      rt = eptmp.tile([128, sq], F32, tag="rt", name="rt")
                      nc.vector.reciprocal(rt[64:128, :], pvA[64:128, :])
                      nc.vector.reciprocal(rt[0:64, :], pvB[0:64, :])
                      rs = eptmp.tile([128, sq], F32, tag="rs", name="rs")
                      nc.sync.dma_start(rs[0:64, :], rt[64:128, :])   # shift
                      nc.sync.dma_start(rs[64:128, :], rt[0:64, :])   # shift
                      nc.vector.tensor_mul(at[0:64, :], pvA[0:64, :],
                                           rs[0:64, :])
                      nc.vector.tensor_mul(
                          at[64:128, :], pvB[64:128, :], rs[64:128, :])

              while pending:
                  pending.popleft()()

              # ---- out projection + residual + layernorm ------------
              if "C" in phases and "B" in phases:
                  for st in range(n_st):
                      ss = slice(st * 128, (st + 1) * 128)
                      x_sb = xtiles.tile([128, D], F32, tag="x")
                      nc.sync.dma_start(x_sb, resid[ss, :])
                      for dt in range(2):
                          ps = proj_ps()
                          for pr in range(NPAIR):
                              nc.tensor.matmul(
                                  ps,
                                  lhsT=(attnT[pr][:, ss]),
                                  rhs=(wo_sb[:, pr, dt * 512:(dt + 1) * 512]),
                                  start=(pr == 0),
                                  stop=(pr == NPAIR - 1),
                              )
                          dsl = slice(dt * 512, (dt + 1) * 512)
                          nc.vector.tensor_add(x_sb[:, dsl], ps, x_sb[:, dsl])
                      # layernorm over D (free dim)
                      stt = stats_pool.tile([128, 2, 6], F32, tag="bst")
                      nc.vector.bn_stats(stt[:, 0, :], x_sb[:, 0:512])
                      nc.vector.bn_stats(stt[:, 1, :], x_sb[:, 512:1024])
                      mv = stats_pool.tile([128, 2], F32, tag="mv")
                      nc.vector.bn_aggr(mv, stt)
                      std = stats_pool.tile([128, 1], F32, tag="std")
                      nc.scalar.activation(
                          std, mv[:, 1:2], AF.Sqrt, bias=eps_sb[:, 0:1])
                      rstd = stats_pool.tile([128, 1], F32, tag="rstd")
                      nc.vector.reciprocal(rstd, std)
                      nc.vector.tensor_scalar(
                          x_sb, x_sb,
                          scalar1=mv[:, 0:1], scalar2=rstd,
                          op0=mybir.AluOpType.subtract,
                          op1=mybir.AluOpType.mult,
                      )
                      nc.vector.tensor_mul(x_sb, x_sb, lng_sb)
                      nc.vector.tensor_add(x_sb, x_sb, lnb_sb)
                      nc.gpsimd.dma_start(out[ss, :], x_sb)

    return nc


_CACHED = {}


def _get_program(sq=SQ, skv=S, repeat=1, phases="ABC"):
    key = (sq, skv, repeat, phases)
    if key not in _CACHED:
        nc = bacc.Bacc("TRN2", target_bir_lowering=False, debug=False)
        build_core_program(nc, sq, skv, repeat, phases)
        nc.finalize()
        _CACHED[key] = nc
    return _CACHED[key]


def make_in_maps(q, k, v, Wq, bq, Wk, bk, Wv, bv, Wo, bo, ln_g, ln_b):
    f = np.float32
    bf = mybir.dt.np(BF16)
    shared = {
        "wqT": np.ascontiguousarray(Wq.T).astype(bf),
        "wkT": np.ascontiguousarray(Wk.T).astype(bf),
        "wvT": np.ascontiguousarray(Wv.T).astype(bf),
        "woT": np.ascontiguousarray(Wo.T).astype(bf),
        "bq": np.ascontiguousarray(bq, f),
        "bk": np.ascontiguousarray(bk, f),
        "bv": np.ascontiguousarray(bv, f),
        "lng": np.ascontiguousarray(ln_g, f),
        "lnb": np.ascontiguousarray(ln_b, f),
    }
    in_maps = []
    for c in range(NCORES):
        b, half = c // 2, c % 2
        rows = slice(half * SQ, (half + 1) * SQ)
        in_maps.append({
            **shared,
            "qT": np.ascontiguousarray(q[b, rows, :].T).astype(bf),
            "kT": np.ascontiguousarray(k[b].T).astype(bf),
            "vT": np.ascontiguousarray(v[b].T).astype(bf),
            "resid": np.ascontiguousarray(q[b, rows, :] + bo[None, :], f),
        })
    return in_maps


def kernel(q, k, v, mask, Wq, bq, Wk, bk, Wv, bv, Wo, bo, ln_g, ln_b):
    nc = _get_program()
    in_maps = make_in_maps(q, k, v, Wq, bq, Wk, bk, Wv, bv, Wo, bo, ln_g, ln_b)
    res = run_bass_kernel_spmd(nc, in_maps, core_ids=list(range(NCORES)))
    out = np.empty((B, S, D), np.float32)
    for c in range(NCORES):
        b, half = c // 2, c % 2
        out[b, half * SQ:(half + 1) * SQ, :] = res.results[c]["out"]
    return out


# revision 16
# speedup vs baseline: 1.8781x; 1.2199x over previous
"""Multi-head attention + residual + layernorm kernel for 8 Trainium2 cores.

Reference computation (B=4, S=2048, D=1024, H=16, dk=64):
    qh,kh,vh = split_heads(x @ W{q,k,v}.T + b)   per batch
    attn     = softmax(qh @ kh^T / 8) @ vh       (mask all-ones)
    out      = LN(concat(attn) @ Wo.T + bo + q)

Sharding: core c -> (batch b = c//2, query rows half = c%2). Each core
computes all 16 heads for its 1024 query rows, using the full 2048 K/V
rows of its batch. No collectives; host concatenates the 8 output shards.

v2 design (vs the phase-serial v1):
  - all matmul operands in bf16 (same PE rate as fp32r, half the DMA/SBUF)
  - khT / vh / va live entirely in SBUF: no DRAM staging round-trip
  - K/Q projections are emitted as "proj blocks" interleaved into the
    attention kc-loop of the PREVIOUS pair, filling the PE gaps that the
    ACT-paced softmax leaves; V projection runs in two dout-half passes
    feeding pairs 0-3 / 4-7.
  - PSUM: scores + proj share one 2-slot tag (4 banks), PV holds the
    other 4 banks -> exactly 8.
  - attention per pair: scores^T via kh/qh partition-split (heads A/B on
    PE row groups 0:64 / 64:128), exp on ACT with fused 1/8 scale, PV
    with [vh | ones] augmented stationaries giving attn^T and the
    softmax denominator in one accumulation.
"""

from collections import deque

import numpy as np

import concourse.bass as bass
import concourse.mybir as mybir
import concourse.tile as tile
from concourse import bacc
from concourse.bass_utils import run_bass_kernel_spmd

F32 = mybir.dt.float32
BF16 = mybir.dt.bfloat16
FP8 = mybir.dt.float8e4
DR = mybir.MatmulPerfMode.DoubleRow
AF = mybir.ActivationFunctionType

B, S, D, H = 4, 2048, 1024, 16
DK = D // H          # 64
NCORES = 8
SQ = S // 2          # query rows per core = 1024
NPAIR = 8            # head pairs; pair p = heads (2p, 2p+1), douts 128p..+128
CH = D // 128        # 8 contraction chunks of 128
LNEPS = 1e-5


def build_core_program(nc, sq=SQ, skv=S, repeat=1, phases='ABC'):
    """Emit the per-core program. sq/skv parameterized only for mini-tests."""
    kcn = skv // 128      # attention key chunks (16)
    n_sq_t = sq // 512    # q 512-tiles (2)
    n_skv_t = skv // 512  # kv 512-tiles (4)
    n_vs_t = skv // 128   # v s-chunks of 128 (16)
    n_st = sq // 128      # out s-tiles (8)
    nqt = sq // 512       # q 512-tiles inside attention (2)

    def din(name, shape, dt=F32):
        return nc.dram_tensor(name, shape, dt, kind="ExternalInput").ap()

    qT = din("qT", [D, sq], FP8)       # this core's q rows, transposed
    kT = din("kT", [D, skv], FP8)
    vT = din("vT", [D, skv], FP8)
    wqT = din("wqT", [D, D], FP8)      # Wq.T etc. ([din, dout])
    wkT = din("wkT", [D, D], FP8)
    wvT = din("wvT", [D, D], FP8)
    woT = din("woT", [D, D], BF16)
    bq = din("bq", [D])
    bk = din("bk", [D])
    bv = din("bv", [D])
    resid = din("resid", [sq, D])  # q rows + bo (host precomputed)
    lng = din("lng", [D])
    lnb = din("lnb", [D])
    out = nc.dram_tensor("out", [sq, D], F32, kind="ExternalOutput").ap()

    with tile.TileContext(nc) as tc:
        with (
            tc.tile_pool(name="consts", bufs=1) as consts,
            tc.tile_pool(name="weights", bufs=1) as weights,
            tc.tile_pool(name="acts", bufs=1) as acts,
            tc.tile_pool(name="vtp", bufs=2) as vtp,
            tc.tile_pool(name="vh", bufs=2) as vh_pool,
            tc.tile_pool(name="khT", bufs=2) as khT_pool,
            tc.tile_pool(name="qht", bufs=2) as qht_pool,
            tc.tile_pool(name="va_pool", bufs=3) as va_pool,
            tc.tile_pool(name="exps", bufs=3) as exps,
            tc.tile_pool(name="attnT", bufs=NPAIR) as attnT_pool,
            tc.tile_pool(name="eptmp", bufs=1) as eptmp,
            tc.tile_pool(name="xtiles", bufs=2) as xtiles,
            tc.tile_pool(name="stats", bufs=4) as stats_pool,
        ):
            # ---- constants -----------------------------------------
            # per-dout bias, striped so dout = pair*128 + p -> [p, pair]
            bq_sb = consts.tile([128, NPAIR], F32)
            nc.scalar.dma_start(bq_sb, bq.rearrange("(pr p) -> p pr", p=128))
            bk_sb = consts.tile([128, NPAIR], F32)
            nc.scalar.dma_start(bk_sb, bk.rearrange("(pr p) -> p pr", p=128))
            bv_sb = consts.tile([128, D], F32)
            nc.scalar.dma_start(bv_sb, bv[None, :].to_broadcast((128, D)))
            lng_sb = consts.tile([128, D], F32)
            nc.scalar.dma_start(lng_sb, lng[None, :].to_broadcast((128, D)))
            lnb_sb = consts.tile([128, D], F32)
            nc.scalar.dma_start(lnb_sb, lnb[None, :].to_broadcast((128, D)))
            eps_sb = consts.tile([128, 1], F32)
            nc.vector.memset(eps_sb, LNEPS)
            ones_sb = consts.tile([128, DK], BF16)
            nc.vector.memset(ones_sb, 1.0)

            for _rep in range(repeat):
             with tc.tile_pool(name=f"psum{_rep}", bufs=2, space="PSUM") \
                     as psum_pool:
              def proj_ps():
                  return psum_pool.tile([128, sq], F32, tag="sc",
                                        name="projps")[:, 0:512]

              # ---- bulk input loads ---------------------------------
              # fp8 DoubleRow layout: din = c*256 + j*128 + p -> [p, c, j, .]
              CH2 = CH // 2
              wk_sb = weights.tile([128, CH2, 2, D], FP8, tag="wk")
              nc.sync.dma_start(
                  wk_sb, wkT.rearrange("(c j p) m -> p c j m", p=128, j=2))
              kt_st = []
              for st in range(n_skv_t):
                  t = acts.tile([128, CH2, 2, 512], FP8, tag=f"kt{st}")
                  nc.scalar.dma_start(
                      t, kT.rearrange("(c j p) s -> p c j s", p=128, j=2)[
                          :, :, :, st * 512:(st + 1) * 512])
                  kt_st.append(t)
              wq_sb = weights.tile([128, CH2, 2, D], FP8, tag="wq")
              nc.sync.dma_start(
                  wq_sb, wqT.rearrange("(c j p) m -> p c j m", p=128, j=2))
              qt_st = []
              for st in range(n_sq_t):
                  t = acts.tile([128, CH2, 2, 512], FP8, tag=f"qt{st}")
                  nc.scalar.dma_start(
                      t, qT.rearrange("(c j p) s -> p c j s", p=128, j=2)[
                          :, :, :, st * 512:(st + 1) * 512])
                  qt_st.append(t)
              wv_sb = weights.tile([128, CH2, 2, D], FP8, tag="wvo")
              nc.sync.dma_start(
                  wv_sb, wvT.rearrange("(c j p) m -> p c j m", p=128, j=2))
              vt_all = []
              for vh_ in range(2):
                  t = acts.tile([128, CH2, 2, skv // 2], FP8, tag=f"vt{vh_}")
                  nc.sync.dma_start(
                      t, vT.rearrange("(c j p) s -> p c j s", p=128, j=2)[
                          :, :, :, vh_ * (skv // 2):(vh_ + 1) * (skv // 2)])
                  vt_all.append(t)

              # ---- proj building blocks -----------------------------
              khT_tiles = [None] * NPAIR
              qht_tiles = [None] * NPAIR
              vh_half = [None, None]  # [128 key-part, kcn, 512 douts] bf16

              def kproj_block(pr, st):
                  def emit():
                      ps = proj_ps()
                      for c in range(CH2):
                          nc.tensor.matmul(
                              ps,
                              lhsT=(wk_sb[:, c, :, pr * 128:(pr + 1) * 128]),
                              rhs=(kt_st[st][:, c, :, :]),
                              start=(c == 0), stop=(c == CH2 - 1),
                              perf_mode=DR,
                          )
                      nc.vector.tensor_scalar_add(
                          khT_tiles[pr][:, st * 512:(st + 1) * 512], ps,
                          scalar1=bk_sb[:, pr:pr + 1])
                  return emit

              def qproj_block(pr, st):
                  def emit():
                      ps = proj_ps()
                      for c in range(CH2):
                          nc.tensor.matmul(
                              ps,
                              lhsT=(wq_sb[:, c, :, pr * 128:(pr + 1) * 128]),
                              rhs=(qt_st[st][:, c, :, :]),
                              start=(c == 0), stop=(c == CH2 - 1),
                              perf_mode=DR,
                          )
                      nc.vector.tensor_scalar_add(
                          qht_tiles[pr][:, st * 512:(st + 1) * 512], ps,
                          scalar1=bq_sb[:, pr:pr + 1])
                  return emit

              def vproj_block(st, dt):
                  def emit():
                      sh = st // (n_vs_t // 2)       # which vt/vh half tile
                      sl = st % (n_vs_t // 2)
                      ps = proj_ps()
                      for c in range(CH2):
                          nc.tensor.matmul(
                              ps,
                              lhsT=(vt_all[sh][:, c, :,
                                               sl * 128:(sl + 1) * 128]),
                              rhs=(wv_sb[:, c, :, dt * 512:(dt + 1) * 512]),
                              start=(c == 0), stop=(c == CH2 - 1),
                              perf_mode=DR,
                          )
                      nc.vector.tensor_add(
                          vh_half[dt][sh][:, sl, :], ps,
                          bv_sb[:, dt * 512:(dt + 1) * 512])
                  return emit

              def emit_kq(pr):
                  khT_tiles[pr] = khT_pool.tile([128, skv], BF16, tag="khT",
                                                name="khT")
                  qht_tiles[pr] = qht_pool.tile([128, sq], BF16, tag="qht",
                                                name="qht")
                  blocks = [kproj_block(pr, st) for st in range(n_skv_t)]
                  blocks += [qproj_block(pr, st) for st in range(n_sq_t)]
                  return blocks

              def emit_vhalf(dt):
                  vh_half[dt] = vh_pool.tile([128, kcn, 512], BF16,
                                             tag="vh", name="vh")
                  return [vproj_block(st, dt) for st in range(n_vs_t)]

              # work queue of pending proj blocks, drained inside the
              # attention kc loops to fill PE gaps
              pending = deque()

              def pop_blocks(n):
                  for _ in range(n):
                      if pending:
                          pending.popleft()()

              # prefix: pair 0's K/Q proj + V douts 0:512 run up front
              if "A" in phases:
                  for b_ in emit_kq(0):
                      b_()
                  for b_ in emit_vhalf(0):
                      b_()

              # ---- attention per head-pair --------------------------
              attnT = []
              wo_sb = None
              if "B" in phases:
                  for pr in range(NPAIR):
                      if pr + 1 < NPAIR and "A" in phases:
                          pending.extend(emit_kq(pr + 1))
                      if pr == 1 and "A" in phases:
                          pending.extend(emit_vhalf(1))
                      if pr == 5:
                          # out-projection weights (reuses wv's slot, which
                          # frees once the last vproj block has run)
                          wo_sb = weights.tile([128, CH, D], BF16, tag="wvo")
                          nc.sync.dma_start(
                              wo_sb,
                              woT.rearrange("(c p) m -> p c m", p=128))

                      kh_sb = khT_tiles[pr]
                      qh_sb = qht_tiles[pr]
                      vh = vh_half[pr // 4]
                      off = (pr % 4) * 128
                      # augmented PV stationary tiles:
                      # head A (even): [vh | ones] -> rows 0:64 attnT, 64:128 sum
                      # head B (odd):  [ones | vh] -> rows 0:64 sum, 64:128 attnT
                      vaA = va_pool.tile([128, kcn, 128], BF16, tag="va")
                      nc.vector.tensor_copy(
                          out=vaA[:, :, 0:DK], in_=vh[:, :, off:off + DK])
                      nc.vector.tensor_copy(
                          out=vaA[:, :, DK:128],
                          in_=ones_sb[:, None, :].to_broadcast((128, kcn, DK)))
                      vaB = va_pool.tile([128, kcn, 128], BF16, tag="va")
                      nc.vector.tensor_copy(
                          out=vaB[:, :, DK:128],
                          in_=vh[:, :, off + DK:off + 128])
                      nc.vector.tensor_copy(
                          out=vaB[:, :, 0:DK],
                          in_=ones_sb[:, None, :].to_broadcast((128, kcn, DK)))

                      pvA = psum_pool.tile([128, sq], F32, tag="pv")
                      pvB = psum_pool.tile([128, sq], F32, tag="pv")

                      # software-pipelined: scores(kc+1) is emitted BEFORE
                      # PV(kc) so the in-order PE queue never stalls behind a
                      # PV matmul that waits on exp(kc) (ACT); steady state
                      # runs PE [scores(kc+1), PV(kc)] || ACT [exp(kc)].
                      def emit_scores(kc):
                          ksl = slice(kc * 128, (kc + 1) * 128)
                          sc = psum_pool.tile([128, sq], F32, tag="sc",
                                              name="sc")
                          scB = psum_pool.tile([128, sq], F32, tag="sc",
                                               name="scB")
                          for qt in range(nqt):
                              qs = slice(qt * 512, (qt + 1) * 512)
                              # head A (rows 0:64) and head B (rows 64:128)
                              # land on different PE row groups -> concurrent
                              nc.tensor.matmul(
                                  sc[:, qs],
                                  lhsT=(kh_sb[0:DK, ksl]),
                                  rhs=(qh_sb[0:DK, qs]),
                                  start=True, stop=True,
                              )
                              nc.tensor.matmul(
                                  scB[:, qs],
                                  lhsT=(kh_sb[DK:128, ksl]),
                                  rhs=(qh_sb[DK:128, qs]),
                                  start=True, stop=True,
                              )
                          return sc, scB

                      sc_next = emit_scores(0)
                      for kc in range(kcn):
                          sc, scB = sc_next
                          sc_next = emit_scores(kc + 1) if kc + 1 < kcn else None
                          ex = exps.tile([128, sq], BF16, tag="ex", name="ex")
                          exB = exps.tile([128, sq], BF16, tag="ex", name="exB")
                          nc.scalar.activation(ex, sc, AF.Exp,
                                               scale=1.0 / np.sqrt(DK))
                          nc.scalar.activation(exB, scB, AF.Exp,
                                               scale=1.0 / np.sqrt(DK))
                          for qt in range(nqt):
                              qs = slice(qt * 512, (qt + 1) * 512)
                              nc.tensor.matmul(
                                  pvA[:, qs], lhsT=(vaA[:, kc, :]),
                                  rhs=(ex[:, qs]),
                                  start=(kc == 0), stop=(kc == kcn - 1),
                              )
                              nc.tensor.matmul(
                                  pvB[:, qs], lhsT=(vaB[:, kc, :]),
                                  rhs=(exB[:, qs]),
                                  start=(kc == 0), stop=(kc == kcn - 1),
                              )
                          pop_blocks(1 + (len(pending) > 6))

                      # epilogue: attnT[0:64] = pvA[0:64] * 1/sumA (sumA on
                      # pvA[64:128]); attnT[64:128] = pvB[64:128] * 1/sumB
                      at = attnT_pool.tile([128, sq], BF16, tag="attnT",
                                           name="attnT")
                      attnT.append(at)
                      rt = eptmp.tile([128, sq], F32, tag="rt", name="rt")
                      nc.vector.reciprocal(rt[64:128, :], pvA[64:128, :])
                      nc.vector.reciprocal(rt[0:64, :], pvB[0:64, :])
                      rs = eptmp.tile([128, sq], F32, tag="rs", name="rs")
                      nc.sync.dma_start(rs[0:64, :], rt[64:128, :])   # shift
                      nc.sync.dma_start(rs[64:128, :], rt[0:64, :])   # shift
                      nc.vector.tensor_mul(at[0:64, :], pvA[0:64, :],
                                           rs[0:64, :])
                      nc.vector.tensor_mul(
                          at[64:128, :], pvB[64:128, :], rs[64:128, :])

              while pending:
                  pending.popleft()()

              # ---- out projection + residual + layernorm ------------
              if "C" in phases and "B" in phases:
                  for st in range(n_st):
                      ss = slice(st * 128, (st + 1) * 128)
                      x_sb = xtiles.tile([128, D], F32, tag="x")
                      nc.sync.dma_start(x_sb, resid[ss, :])
                      for dt in range(2):
                          ps = proj_ps()
                          for pr in range(NPAIR):
                              nc.tensor.matmul(
                                  ps,
                                  lhsT=(attnT[pr][:, ss]),
                                  rhs=(wo_sb[:, pr, dt * 512:(dt + 1) * 512]),
                                  start=(pr == 0),
                                  stop=(pr == NPAIR - 1),
                              )
                          dsl = slice(dt * 512, (dt + 1) * 512)
                          nc.vector.tensor_add(x_sb[:, dsl], ps, x_sb[:, dsl])
                      # layernorm over D (free dim)
                      stt = stats_pool.tile([128, 2, 6], F32, tag="bst")
                      nc.vector.bn_stats(stt[:, 0, :], x_sb[:, 0:512])
                      nc.vector.bn_stats(stt[:, 1, :], x_sb[:, 512:1024])
                      mv = stats_pool.tile([128, 2], F32, tag="mv")
                      nc.vector.bn_aggr(mv, stt)
                      std = stats_pool.tile([128, 1], F32, tag="std")
                      nc.scalar.activation(
                          std, mv[:, 1:2], AF.Sqrt, bias=eps_sb[:, 0:1])
                      rstd = stats_pool.tile([128, 1], F32, tag="rstd")
                      nc.vector.reciprocal(rstd, std)
                      nc.vector.tensor_scalar(
                          x_sb, x_sb,
                          scalar1=mv[:, 0:1], scalar2=rstd,
                          op0=mybir.AluOpType.subtract,
                          op1=mybir.AluOpType.mult,
                      )
                      nc.vector.tensor_mul(x_sb, x_sb, lng_sb)
                      nc.vector.tensor_add(x_sb, x_sb, lnb_sb)
                      nc.gpsimd.dma_start(out[ss, :], x_sb)

    return nc


_CACHED = {}


def _get_program(sq=SQ, skv=S, repeat=1, phases="ABC"):
    key = (sq, skv, repeat, phases)
    if key not in _CACHED:
        nc = bacc.Bacc("TRN2", target_bir_lowering=False, debug=False)
        build_core_program(nc, sq, skv, repeat, phases)
        nc.finalize()
        _CACHED[key] = nc
    return _CACHED[key]


def make_in_maps(q, k, v, Wq, bq, Wk, bk, Wv, bv, Wo, bo, ln_g, ln_b):
    f = np.float32
    bf = mybir.dt.np(BF16)
    f8 = mybir.dt.np(FP8)
    shared = {
        "wqT": np.ascontiguousarray(Wq.T).astype(f8),
        "wkT": np.ascontiguousarray(Wk.T).astype(f8),
        "wvT": np.ascontiguousarray(Wv.T).astype(f8),
        "woT": np.ascontiguousarray(Wo.T).astype(bf),
        "bq": np.ascontiguousarray(bq, f),
        "bk": np.ascontiguousarray(bk, f),
        "bv": np.ascontiguousarray(bv, f),
        "lng": np.ascontiguousarray(ln_g, f),
        "lnb": np.ascontiguousarray(ln_b, f),
    }
    in_maps = []
    for c in range(NCORES):
        b, half = c // 2, c % 2
        rows = slice(half * SQ, (half + 1) * SQ)
        in_maps.append({
            **shared,
            "qT": np.ascontiguousarray(q[b, rows, :].T).astype(f8),
            "kT": np.ascontiguousarray(k[b].T).astype(f8),
            "vT": np.ascontiguousarray(v[b].T).astype(f8),
            "resid": np.ascontiguousarray(q[b, rows, :] + bo[None, :], f),
        })
    return in_maps


def kernel(q, k, v, mask, Wq, bq, Wk, bk, Wv, bv, Wo, bo, ln_g, ln_b):
    nc = _get_program()
    in_maps = make_in_maps(q, k, v, Wq, bq, Wk, bk, Wv, bv, Wo, bo, ln_g, ln_b)
    res = run_bass_kernel_spmd(nc, in_maps, core_ids=list(range(NCORES)))
    out = np.empty((B, S, D), np.float32)
    for c in range(NCORES):
        b, half = c // 2, c % 2
        out[b, half * SQ:(half + 1) * SQ, :] = res.results[c]["out"]
    return out


# revision 30
# speedup vs baseline: 2.0905x; 1.1131x over previous
"""Multi-head attention + residual + layernorm kernel for 8 Trainium2 cores.

Reference computation (B=4, S=2048, D=1024, H=16, dk=64):
    qh,kh,vh = split_heads(x @ W{q,k,v}.T + b)   per batch
    attn     = softmax(qh @ kh^T / 8) @ vh       (mask all-ones)
    out      = LN(concat(attn) @ Wo.T + bo + q)

Sharding: core c -> (batch b = c//2, query rows half = c%2). Each core
computes all 16 heads for its 1024 query rows, using the full 2048 K/V
rows of its batch. No collectives; host concatenates the 8 output shards.

v2 design (vs the phase-serial v1):
  - all matmul operands in bf16 (same PE rate as fp32r, half the DMA/SBUF)
  - khT / vh / va live entirely in SBUF: no DRAM staging round-trip
  - K/Q projections are emitted as "proj blocks" interleaved into the
    attention kc-loop of the PREVIOUS pair, filling the PE gaps that the
    ACT-paced softmax leaves; V projection runs in two dout-half passes
    feeding pairs 0-3 / 4-7.
  - PSUM: scores + proj share one 2-slot tag (4 banks), PV holds the
    other 4 banks -> exactly 8.
  - attention per pair: scores^T via kh/qh partition-split (heads A/B on
    PE row groups 0:64 / 64:128), exp on ACT with fused 1/8 scale, PV
    with [vh | ones] augmented stationaries giving attn^T and the
    softmax denominator in one accumulation.
"""

from collections import deque

import numpy as np

import concourse.bass as bass
import concourse.mybir as mybir
import concourse.tile as tile
from concourse import bacc
from concourse.bass_utils import run_bass_kernel_spmd

F32 = mybir.dt.float32
BF16 = mybir.dt.bfloat16
FP8 = mybir.dt.float8e4
DR = mybir.MatmulPerfMode.DoubleRow
AF = mybir.ActivationFunctionType

B, S, D, H = 4, 2048, 1024, 16
DK = D // H          # 64
NCORES = 8
SQ = S // 2          # query rows per core = 1024
NPAIR = 8            # head pairs; pair p = heads (2p, 2p+1), douts 128p..+128
CH = D // 128        # 8 contraction chunks of 128
LNEPS = 1e-5


def build_core_program(nc, sq=SQ, skv=S, repeat=1, phases='ABC'):
    """Emit the per-core program. sq/skv parameterized only for mini-tests."""
    kcn = skv // 128      # attention key chunks (16)
    n_sq_t = sq // 512    # q 512-tiles (2)
    n_skv_t = skv // 512  # kv 512-tiles (4)
    n_vs_t = skv // 128   # v s-chunks of 128 (16)
    n_st = sq // 128      # out s-tiles (8)
    nqt = sq // 512       # q 512-tiles inside attention (2)

    def din(name, shape, dt=F32):
        return nc.dram_tensor(name, shape, dt, kind="ExternalInput").ap()

    qT = din("qT", [D, sq], FP8)       # this core's q rows, transposed
    kT = din("kT", [D, skv], FP8)
    vT = din("vT", [D, skv], FP8)
    wqT = din("wqT", [D, D], FP8)      # Wq.T etc. ([din, dout])
    wkT = din("wkT", [D, D], FP8)
    wvT = din("wvT", [D, D], FP8)
    woT = din("woT", [D, D], BF16)
    bq = din("bq", [D])
    bk = din("bk", [D])
    bv = din("bv", [D])
    resid = din("resid", [sq, D])  # q rows + bo (host precomputed)
    lng = din("lng", [D])
    lnb = din("lnb", [D])
    out = nc.dram_tensor("out", [sq, D], F32, kind="ExternalOutput").ap()

    with tile.TileContext(nc) as tc:
        with (
            tc.tile_pool(name="consts", bufs=1) as consts,
            tc.tile_pool(name="weights", bufs=1) as weights,
            tc.tile_pool(name="acts", bufs=1) as acts,
            tc.tile_pool(name="vtp", bufs=2) as vtp,
            tc.tile_pool(name="vh", bufs=4) as vh_pool,
            tc.tile_pool(name="xp", bufs=n_st) as xp_pool,
            tc.tile_pool(name="khT", bufs=2) as khT_pool,
            tc.tile_pool(name="qht", bufs=2) as qht_pool,
            tc.tile_pool(name="va_pool", bufs=3) as va_pool,
            tc.tile_pool(name="exps", bufs=3) as exps,
            tc.tile_pool(name="attnT", bufs=NPAIR) as attnT_pool,
            tc.tile_pool(name="eptmp", bufs=1) as eptmp,
            tc.tile_pool(name="stats", bufs=4) as stats_pool,
        ):
            # ---- constants -----------------------------------------
            # per-dout bias, striped so dout = pair*128 + p -> [p, pair]
            bq_sb = consts.tile([128, NPAIR], F32)
            nc.scalar.dma_start(bq_sb, bq.rearrange("(pr p) -> p pr", p=128))
            bk_sb = consts.tile([128, NPAIR], F32)
            nc.scalar.dma_start(bk_sb, bk.rearrange("(pr p) -> p pr", p=128))
            bv_sb = consts.tile([128, D], F32)
            nc.scalar.dma_start(bv_sb, bv[None, :].to_broadcast((128, D)))
            lng_sb = consts.tile([128, D], F32)
            nc.scalar.dma_start(lng_sb, lng[None, :].to_broadcast((128, D)))
            lnb_sb = consts.tile([128, D], F32)
            nc.scalar.dma_start(lnb_sb, lnb[None, :].to_broadcast((128, D)))
            eps_sb = consts.tile([128, 1], F32)
            nc.vector.memset(eps_sb, LNEPS)
            ones_sb = consts.tile([128, DK], BF16)
            nc.vector.memset(ones_sb, 1.0)

            for _rep in range(repeat):
             with tc.tile_pool(name=f"psum{_rep}", bufs=2, space="PSUM") \
                     as psum_pool:
              def proj_ps():
                  return psum_pool.tile([128, sq], F32, tag="sc",
                                        name="projps")[:, 0:512]

              # ---- bulk input loads ---------------------------------
              # fp8 DoubleRow layout: din = c*256 + j*128 + p -> [p, c, j, .]
              CH2 = CH // 2
              wk_sb = weights.tile([128, CH2, 2, D], FP8, tag="wk")
              nc.sync.dma_start(
                  wk_sb, wkT.rearrange("(c j p) m -> p c j m", p=128, j=2))
              kt_st = []
              for st in range(n_skv_t):
                  t = acts.tile([128, CH2, 2, 512], FP8, tag=f"kt{st}")
                  nc.sync.dma_start(
                      t, kT.rearrange("(c j p) s -> p c j s", p=128, j=2)[
                          :, :, :, st * 512:(st + 1) * 512])
                  kt_st.append(t)
              wq_sb = weights.tile([128, CH2, 2, D], FP8, tag="wq")
              nc.sync.dma_start(
                  wq_sb, wqT.rearrange("(c j p) m -> p c j m", p=128, j=2))
              qt_st = []
              for st in range(n_sq_t):
                  t = acts.tile([128, CH2, 2, 512], FP8, tag=f"qt{st}")
                  nc.scalar.dma_start(
                      t, qT.rearrange("(c j p) s -> p c j s", p=128, j=2)[
                          :, :, :, st * 512:(st + 1) * 512])
                  qt_st.append(t)
              wv_sb = weights.tile([128, CH2, 2, D], FP8, tag="wvo")
              nc.sync.dma_start(
                  wv_sb, wvT.rearrange("(c j p) m -> p c j m", p=128, j=2))
              vt_all = []
              for vh_ in range(2):
                  t = acts.tile([128, CH2, 2, skv // 2], FP8, tag=f"vt{vh_}")
                  nc.sync.dma_start(
                      t, vT.rearrange("(c j p) s -> p c j s", p=128, j=2)[
                          :, :, :, vh_ * (skv // 2):(vh_ + 1) * (skv // 2)])
                  vt_all.append(t)

              # ---- proj building blocks -----------------------------
              khT_tiles = [None] * NPAIR
              qht_tiles = [None] * NPAIR
              vh_half = [None, None]  # [128 key-part, kcn, 512 douts] bf16

              def kproj_block(pr, st):
                  def emit():
                      ps = proj_ps()
                      for c in range(CH2):
                          nc.tensor.matmul(
                              ps,
                              lhsT=(wk_sb[:, c, :, pr * 128:(pr + 1) * 128]),
                              rhs=(kt_st[st][:, c, :, :]),
                              start=(c == 0), stop=(c == CH2 - 1),
                              perf_mode=DR,
                          )
                      nc.vector.tensor_scalar_add(
                          khT_tiles[pr][:, st * 512:(st + 1) * 512], ps,
                          scalar1=bk_sb[:, pr:pr + 1])
                  return emit

              def qproj_block(pr, st):
                  def emit():
                      ps = proj_ps()
                      for c in range(CH2):
                          nc.tensor.matmul(
                              ps,
                              lhsT=(wq_sb[:, c, :, pr * 128:(pr + 1) * 128]),
                              rhs=(qt_st[st][:, c, :, :]),
                              start=(c == 0), stop=(c == CH2 - 1),
                              perf_mode=DR,
                          )
                      nc.vector.tensor_scalar_add(
                          qht_tiles[pr][:, st * 512:(st + 1) * 512], ps,
                          scalar1=bq_sb[:, pr:pr + 1])
                  return emit

              def vproj_block(st, dt):
                  def emit():
                      sh = st // (n_vs_t // 2)       # which vt/vh half tile
                      sl = st % (n_vs_t // 2)
                      ps = proj_ps()
                      for c in range(CH2):
                          nc.tensor.matmul(
                              ps,
                              lhsT=(vt_all[sh][:, c, :,
                                               sl * 128:(sl + 1) * 128]),
                              rhs=(wv_sb[:, c, :, dt * 512:(dt + 1) * 512]),
                              start=(c == 0), stop=(c == CH2 - 1),
                              perf_mode=DR,
                          )
                      nc.vector.tensor_add(
                          vh_half[dt][sh][:, sl, :], ps,
                          bv_sb[:, dt * 512:(dt + 1) * 512])
                  return emit

              def emit_kq(pr):
                  khT_tiles[pr] = khT_pool.tile([128, skv], BF16, tag="khT",
                                                name="khT")
                  qht_tiles[pr] = qht_pool.tile([128, sq], BF16, tag="qht",
                                                name="qht")
                  blocks = [kproj_block(pr, st) for st in range(n_skv_t)]
                  blocks += [qproj_block(pr, st) for st in range(n_sq_t)]
                  return blocks

              def emit_vhalf(dt):
                  vh_half[dt] = [
                      vh_pool.tile([128, kcn // 2, 512], BF16,
                                   tag="vh", name="vh")
                      for _ in range(2)
                  ]
                  return [vproj_block(st, dt) for st in range(n_vs_t)]

              # out-proj partial sums over pairs 0..6, computed during
              # attn(7) to shrink the serial tail
              xp_tiles = [None] * n_st

              def cpart_block(st, dt):
                  def emit():
                      ss = slice(st * 128, (st + 1) * 128)
                      dsl = slice(dt * 512, (dt + 1) * 512)
                      ps = proj_ps()
                      for pr in range(NPAIR - 1):
                          nc.tensor.matmul(
                              ps,
                              lhsT=(attnT[pr][:, ss]),
                              rhs=(wo_sb[:, pr, dt * 512:(dt + 1) * 512]),
                              start=(pr == 0),
                              stop=(pr == NPAIR - 2),
                          )
                      nc.vector.tensor_add(
                          xp_tiles[st][:, dsl], ps, xp_tiles[st][:, dsl])
                  return emit

              # work queue of pending proj blocks, drained inside the
              # attention kc loops to fill PE gaps
              pending = deque()

              def pop_blocks(n):
                  for _ in range(n):
                      if pending:
                          pending.popleft()()

              # prefix: pair 0's K/Q proj + V douts 0:512 x keys 0:1024 run
              # up front (PV of kc 0-7 only needs the sh0 half of vh); the
              # rest of Vdt0 drains at the head of the pending queue
              vdt0_rest = []
              if "A" in phases:
                  for b_ in emit_kq(0):
                      b_()
                  vdt0 = emit_vhalf(0)
                  for b_ in vdt0[:n_vs_t // 2]:
                      b_()
                  vdt0_rest = vdt0[n_vs_t // 2:]

              # ---- attention per head-pair --------------------------
              attnT = []
              wo_sb = None
              if "B" in phases:
                  for pr in range(NPAIR):
                      if pr == 0:
                          pending.extend(vdt0_rest)
                      if pr + 1 < NPAIR and "A" in phases:
                          pending.extend(emit_kq(pr + 1))
                      if pr == 1 and "A" in phases:
                          pending.extend(emit_vhalf(1))
                      if pr == 5:
                          # out-projection weights (reuses wv's slot, which
                          # frees once the last vproj block has run)
                          wo_sb = weights.tile([128, CH, D], BF16, tag="wvo")
                          nc.sync.dma_start(
                              wo_sb,
                              woT.rearrange("(c p) m -> p c m", p=128))
                      if pr == NPAIR - 2 and "C" in phases:
                          # residual preloads into the out-proj accumulators
                          for st in range(n_st):
                              xp_tiles[st] = xp_pool.tile(
                                  [128, D], F32, tag="xp", name="xp")
                              nc.gpsimd.dma_start(
                                  xp_tiles[st],
                                  resid[st * 128:(st + 1) * 128, :])
                      if pr == NPAIR - 1 and "C" in phases:
                          pending.extend(cpart_block(st, dt)
                                         for st in range(n_st)
                                         for dt in range(2))

                      kh_sb = khT_tiles[pr]
                      qh_sb = qht_tiles[pr]
                      vh = vh_half[pr // 4]
                      off = (pr % 4) * 128
                      kc2 = kcn // 2
                      # augmented PV stationary tiles:
                      # head A (even): [vh | ones] -> rows 0:64 attnT, 64:128 sum
                      # head B (odd):  [ones | vh] -> rows 0:64 sum, 64:128 attnT
                      vaA = va_pool.tile([128, kcn, 128], BF16, tag="va")
                      for sh in range(2):
                          nc.vector.tensor_copy(
                              out=vaA[:, sh * kc2:(sh + 1) * kc2, 0:DK],
                              in_=vh[sh][:, :, off:off + DK])
                      nc.vector.tensor_copy(
                          out=vaA[:, :, DK:128],
                          in_=ones_sb[:, None, :].to_broadcast((128, kcn, DK)))
                      vaB = va_pool.tile([128, kcn, 128], BF16, tag="va")
                      for sh in range(2):
                          nc.vector.tensor_copy(
                              out=vaB[:, sh * kc2:(sh + 1) * kc2, DK:128],
                              in_=vh[sh][:, :, off + DK:off + 128])
                      nc.vector.tensor_copy(
                          out=vaB[:, :, 0:DK],
                          in_=ones_sb[:, None, :].to_broadcast((128, kcn, DK)))

                      pvA = psum_pool.tile([128, sq], F32, tag="pv")
                      pvB = psum_pool.tile([128, sq], F32, tag="pv")

                      # software-pipelined: scores(kc+1) is emitted BEFORE
                      # PV(kc) so the in-order PE queue never stalls behind a
                      # PV matmul that waits on exp(kc) (ACT); steady state
                      # runs PE [scores(kc+1), PV(kc)] || ACT [exp(kc)].
                      def emit_scores(kc):
                          ksl = slice(kc * 128, (kc + 1) * 128)
                          sc = psum_pool.tile([128, sq], F32, tag="sc",
                                              name="sc")
                          scB = psum_pool.tile([128, sq], F32, tag="sc",
                                               name="scB")
                          for qt in range(nqt):
                              qs = slice(qt * 512, (qt + 1) * 512)
                              # head A (rows 0:64) and head B (rows 64:128)
                              # land on different PE row groups -> concurrent
                              nc.tensor.matmul(
                                  sc[:, qs],
                                  lhsT=(kh_sb[0:DK, ksl]),
                                  rhs=(qh_sb[0:DK, qs]),
                                  start=True, stop=True,
                              )
                              nc.tensor.matmul(
                                  scB[:, qs],
                                  lhsT=(kh_sb[DK:128, ksl]),
                                  rhs=(qh_sb[DK:128, qs]),
                                  start=True, stop=True,
                              )
                          return sc, scB

                      sc_next = emit_scores(0)
                      for kc in range(kcn):
                          sc, scB = sc_next
                          sc_next = emit_scores(kc + 1) if kc + 1 < kcn else None
                          ex = exps.tile([128, sq], BF16, tag="ex", name="ex")
                          exB = exps.tile([128, sq], BF16, tag="ex", name="exB")
                          nc.scalar.activation(ex, sc, AF.Exp,
                                               scale=1.0 / np.sqrt(DK))
                          nc.scalar.activation(exB, scB, AF.Exp,
                                               scale=1.0 / np.sqrt(DK))
                          for qt in range(nqt):
                              qs = slice(qt * 512, (qt + 1) * 512)
                              nc.tensor.matmul(
                                  pvA[:, qs], lhsT=(vaA[:, kc, :]),
                                  rhs=(ex[:, qs]),
                                  start=(kc == 0), stop=(kc == kcn - 1),
                              )
                              nc.tensor.matmul(
                                  pvB[:, qs], lhsT=(vaB[:, kc, :]),
                                  rhs=(exB[:, qs]),
                                  start=(kc == 0), stop=(kc == kcn - 1),
                              )
                          pop_blocks(1 + (len(pending) > 6 and pr < NPAIR - 1))

                      # epilogue: attnT[0:64] = pvA[0:64] * 1/sumA (sumA on
                      # pvA[64:128]); attnT[64:128] = pvB[64:128] * 1/sumB
                      at = attnT_pool.tile([128, sq], BF16, tag="attnT",
                                           name="attnT")
                      attnT.append(at)
                      rt = eptmp.tile([128, sq], F32, tag="rt", name="rt")
                      nc.vector.reciprocal(rt[64:128, :], pvA[64:128, :])
                      nc.vector.reciprocal(rt[0:64, :], pvB[0:64, :])
                      # partition-offset operands: attn rows x shifted recip
                      nc.vector.tensor_mul(at[0:64, :], pvA[0:64, :],
                                           rt[64:128, :])
                      nc.vector.tensor_mul(
                          at[64:128, :], pvB[64:128, :], rt[0:64, :])

              while pending:
                  pending.popleft()()

              # ---- out projection + residual + layernorm ------------
              if "C" in phases and "B" in phases:
                  for st in range(n_st):
                      ss = slice(st * 128, (st + 1) * 128)
                      x_sb = xp_tiles[st]
                      for dt in range(2):
                          ps = proj_ps()
                          nc.tensor.matmul(
                              ps,
                              lhsT=(attnT[NPAIR - 1][:, ss]),
                              rhs=(wo_sb[:, NPAIR - 1,
                                         dt * 512:(dt + 1) * 512]),
                              start=True, stop=True,
                          )
                          dsl = slice(dt * 512, (dt + 1) * 512)
                          nc.vector.tensor_add(x_sb[:, dsl], ps, x_sb[:, dsl])
                      # layernorm over D (free dim); lng/lnb on idle GpSimd
                      stt = stats_pool.tile([128, 2, 6], F32, tag="bst")
                      nc.vector.bn_stats(stt[:, 0, :], x_sb[:, 0:512])
                      nc.vector.bn_stats(stt[:, 1, :], x_sb[:, 512:1024])
                      mv = stats_pool.tile([128, 2], F32, tag="mv")
                      nc.vector.bn_aggr(mv, stt)
                      std = stats_pool.tile([128, 1], F32, tag="std")
                      nc.scalar.activation(
                          std, mv[:, 1:2], AF.Sqrt, bias=eps_sb[:, 0:1])
                      rstd = stats_pool.tile([128, 1], F32, tag="rstd")
                      nc.vector.reciprocal(rstd, std)
                      nc.vector.tensor_scalar(
                          x_sb, x_sb,
                          scalar1=mv[:, 0:1], scalar2=rstd,
                          op0=mybir.AluOpType.subtract,
                          op1=mybir.AluOpType.mult,
                      )
                      nc.gpsimd.tensor_mul(x_sb, x_sb, lng_sb)
                      nc.gpsimd.tensor_add(x_sb, x_sb, lnb_sb)
                      nc.sync.dma_start(out[ss, :], x_sb)

    return nc


_CACHED = {}


def _get_program(sq=SQ, skv=S, repeat=1, phases="ABC"):
    key = (sq, skv, repeat, phases)
    if key not in _CACHED:
        nc = bacc.Bacc("TRN2", target_bir_lowering=False, debug=False)
        build_core_program(nc, sq, skv, repeat, phases)
        nc.finalize()
        _CACHED[key] = nc
    return _CACHED[key]


def make_in_maps(q, k, v, Wq, bq, Wk, bk, Wv, bv, Wo, bo, ln_g, ln_b):
    f = np.float32
    bf = mybir.dt.np(BF16)
    f8 = mybir.dt.np(FP8)
    shared = {
        "wqT": np.ascontiguousarray(Wq.T).astype(f8),
        "wkT": np.ascontiguousarray(Wk.T).astype(f8),
        "wvT": np.ascontiguousarray(Wv.T).astype(f8),
        "woT": np.ascontiguousarray(Wo.T).astype(bf),
        "bq": np.ascontiguousarray(bq, f),
        "bk": np.ascontiguousarray(bk, f),
        "bv": np.ascontiguousarray(bv, f),
        "lng": np.ascontiguousarray(ln_g, f),
        "lnb": np.ascontiguousarray(ln_b, f),
    }
    in_maps = []
    for c in range(NCORES):
        b, half = c // 2, c % 2
        rows = slice(half * SQ, (half + 1) * SQ)
        in_maps.append({
            **shared,
            "qT": np.ascontiguousarray(q[b, rows, :].T).astype(f8),
            "kT": np.ascontiguousarray(k[b].T).astype(f8),
            "vT": np.ascontiguousarray(v[b].T).astype(f8),
            "resid": np.ascontiguousarray(q[b, rows, :] + bo[None, :], f),
        })
    return in_maps


def kernel(q, k, v, mask, Wq, bq, Wk, bk, Wv, bv, Wo, bo, ln_g, ln_b):
    nc = _get_program()
    in_maps = make_in_maps(q, k, v, Wq, bq, Wk, bk, Wv, bv, Wo, bo, ln_g, ln_b)
    res = run_bass_kernel_spmd(nc, in_maps, core_ids=list(range(NCORES)))
    out = np.empty((B, S, D), np.float32)
    for c in range(NCORES):
        b, half = c // 2, c % 2
        out[b, half * SQ:(half + 1) * SQ, :] = res.results[c]["out"]
    return out
